# revision 3
# baseline (speedup 1.0000x reference)
"""Trainium2 Bass kernel for nn_CustomLoss_19061064859882.

Custom loss = cross-entropy(y_pred, y_true) - penalty/N where the penalty
uses per-sample p1 = softmax(y_pred)[:, 0] and a per-class weight s[j]
derived from the label histogram.

Strategy (pure data parallel over 8 NeuronCores, full I/O on host):
  - Shard y_pred / y_true along the batch axis: 32768 rows per core.
  - Per core, per 1024-row batch (8 rows per SBUF partition, linear DMA):
      * ScalarE: E = exp(y_pred) with fused row-sum (accum_out) -> sumexp
      * VectorE: one-hot(labels) via is_equal against an iota row,
                 p1 = E[:,0]/sumexp, v = select(label==0, log(p1+eps),
                 log(1-p1+eps))
      * ScalarE: lse = ln(sumexp), log terms
      * TensorE: PSUM-accumulated one-hot matmuls:
            psA[j, c] += sum_{i: y_i=j} y_pred[i, c]   (diag -> picked sum)
            psB[j, 0] += sum_{i: y_i=j} v_i            (penalty per class)
            psB[j, 1] += sum_{i: y_i=j} lse_i          (lse sum)
  - Host: bincount(y_true), combine the 8 per-core [128,130] partials in
    float64, return the f32 scalar loss.
"""

import sys

import numpy as np

if "/opt/trn_rl_repo" not in sys.path:
    sys.path.insert(0, "/opt/trn_rl_repo")

N_CORES = 8
N = 262144
C = 128  # classes
M = N // N_CORES  # rows per core
P = 128  # SBUF partitions
KB = 8  # rows per partition per batch
BATCH_ROWS = P * KB  # 1024
NB = M // BATCH_ROWS  # batches per core
ALPHA = 0.5
BETA = 0.5
EPS = 1e-9

_CACHE: dict = {}


def _build_nc():
    import concourse.bacc as bacc
    import concourse.mybir as mybir
    import concourse.tile as tile

    f32 = mybir.dt.float32
    i32 = mybir.dt.int32
    Ln = mybir.ActivationFunctionType.Ln
    Exp = mybir.ActivationFunctionType.Exp
    is_eq = mybir.AluOpType.is_equal

    nc = bacc.Bacc(
        "TRN2", target_bir_lowering=False, debug=False, num_devices=N_CORES
    )
    y = nc.dram_tensor("y_pred", [M, C], f32, kind="ExternalInput").ap()
    lab = nc.dram_tensor("labels", [P, NB, KB], f32, kind="ExternalInput").ap()
    out = nc.dram_tensor("out", [P, C + 2], f32, kind="ExternalOutput").ap()

    # row(b, p, k) = b*1024 + p*8 + k  (4KB contiguous per partition per batch)
    y4 = y.rearrange("(b p k) c -> b p k c", b=NB, p=P, k=KB)

    with tile.TileContext(nc) as tc:
        with (
            tc.tile_pool(name="const", bufs=1) as constp,
            tc.tile_pool(name="work", bufs=3) as work,
            tc.tile_pool(name="small", bufs=3) as small,
            tc.tile_pool(name="psum", bufs=1, space="PSUM") as psump,
        ):
            iota_i = constp.tile([P, C], i32)
            nc.gpsimd.iota(iota_i[:], pattern=[[1, C]], channel_multiplier=0)
            iota_f = constp.tile([P, C], f32)
            nc.vector.tensor_copy(iota_f[:], iota_i[:])
            labs = constp.tile([P, NB, KB], f32)
            nc.sync.dma_start(labs[:], lab[:])
            ebias = constp.tile([P, 1], f32)
            nc.gpsimd.memset(ebias[:], EPS)
            obias = constp.tile([P, 1], f32)
            nc.gpsimd.memset(obias[:], 1.0 + EPS)

            psA = psump.tile([P, C], f32)  # per-class sums of y_pred columns
            psB = psump.tile([P, 2], f32)  # [V_j, L_j]

            for b in range(NB):
                T = work.tile([P, KB, C], f32)
                nc.sync.dma_start(T[:], y4[b])
                E = work.tile([P, KB, C], f32)
                O = work.tile([P, KB, C], f32)
                cmb = work.tile([P, KB, 2], f32)
                se = small.tile([P, KB], f32)
                for k in range(KB):
                    nc.scalar.activation(
                        E[:, k, :], T[:, k, :], Exp, accum_out=se[:, k : k + 1]
                    )
                rs = small.tile([P, KB], f32)
                nc.vector.reciprocal(rs[:], se[:])
                p1 = small.tile([P, KB], f32)
                nc.vector.tensor_mul(p1[:], E[:, :, 0], rs[:])
                lp = small.tile([P, KB], f32)
                nc.scalar.activation(lp[:], p1[:], Ln, bias=ebias[:])
                lq = small.tile([P, KB], f32)
                nc.scalar.activation(lq[:], p1[:], Ln, bias=obias[:], scale=-1.0)
                nc.scalar.activation(cmb[:, :, 1], se[:], Ln)  # lse
                for k in range(KB):
                    nc.vector.tensor_scalar(
                        O[:, k, :], iota_f[:], labs[:, b, k : k + 1], None, is_eq
                    )
                d = small.tile([P, KB], f32)
                nc.vector.tensor_sub(d[:], lp[:], lq[:])
                v1 = small.tile([P, KB], f32)
                nc.vector.tensor_mul(v1[:], O[:, :, 0], d[:])
                nc.vector.tensor_add(cmb[:, :, 0], v1[:], lq[:])
                for k in range(KB):
                    first = b == 0 and k == 0
                    last = b == NB - 1 and k == KB - 1
                    nc.tensor.matmul(
                        psB[:],
                        O[:, k, :],
                        cmb[:, k, :],
                        start=first,
                        stop=last,
                        skip_group_check=True,
                    )
                    nc.tensor.matmul(
                        psA[:],
                        O[:, k, :],
                        T[:, k, :],
                        start=first,
                        stop=last,
                        skip_group_check=True,
                    )

            outsb = constp.tile([P, C + 2], f32)
            nc.vector.tensor_copy(outsb[:, 0:C], psA[:])
            nc.vector.tensor_copy(outsb[:, C : C + 2], psB[:])
            nc.sync.dma_start(out[:], outsb[:])

    nc.finalize()
    return nc


def _get_nc():
    if "nc" not in _CACHE:
        _CACHE["nc"] = _build_nc()
    return _CACHE["nc"]


def _make_in_maps(y_pred: np.ndarray, y_true: np.ndarray):
    yp = np.ascontiguousarray(np.asarray(y_pred), dtype=np.float32)
    yt = np.asarray(y_true).reshape(-1)
    in_maps = []
    for c in range(N_CORES):
        ys = yp[c * M : (c + 1) * M]
        lt = (
            yt[c * M : (c + 1) * M]
            .astype(np.float32)
            .reshape(NB, P, KB)
            .transpose(1, 0, 2)
        )
        in_maps.append({"y_pred": ys, "labels": np.ascontiguousarray(lt)})
    return in_maps


def _run(in_maps, trace=False, **kwargs):
    from concourse.bass_utils import run_bass_kernel_spmd

    nc = _get_nc()
    return run_bass_kernel_spmd(
        nc, in_maps, list(range(N_CORES)), trace=trace, **kwargs
    )


def _combine(results, y_true: np.ndarray) -> np.ndarray:
    yt = np.asarray(y_true).reshape(-1).astype(np.int64)
    Pj = np.zeros(C, dtype=np.float64)
    Vj = np.zeros(C, dtype=np.float64)
    Lsum = 0.0
    for c in range(N_CORES):
        o = results[c]["out"].astype(np.float64)
        Pj += np.diagonal(o[:, 0:C])
        Vj += o[:, C]
        Lsum += o[:, C + 1].sum()
    nj = np.bincount(yt, minlength=C).astype(np.float64)
    ce = -(Pj.sum() - Lsum) / N
    s = BETA * (1.0 - nj / (N - nj[0]))
    penalty = ALPHA * Vj[0] + float((s[1:] * Vj[1:]).sum())
    loss = ce - penalty / N
    return np.asarray(loss, dtype=np.float32)


def kernel(y_pred: np.ndarray, y_true: np.ndarray) -> np.ndarray:
    in_maps = _make_in_maps(y_pred, y_true)
    res = _run(in_maps, trace=False)
    return _combine(res.results, y_true)


# revision 4
# speedup vs baseline: 3.8281x; 3.8281x over previous
"""Trainium2 Bass kernel for nn_CustomLoss_19061064859882.

loss = CE(y_pred, y_true) - penalty/N, where the penalty uses
p1 = softmax(y_pred)[:, 0] and per-class weights from the label histogram.

Device/host split: everything that is O(N*C) transcendental work — the
per-row logsumexp over the 128 classes — runs on the 8 NeuronCores
(data-parallel over rows, fp16 on the wire, exp on ScalarE + row-reduce on
VectorE). The remaining O(N) bookkeeping (picked-logit gather, label
bincount, per-class weighted sums, final scalar) is cheap vectorized numpy
on the host, done in float64:

    lse_i   = log(sum_c exp(y_pred[i, c]))          # device
    CE      = -(sum_i y_pred[i, y_i] - sum_i lse_i)/N
    p1_i    = exp(y_pred[i, 0] - lse_i)
    v_i     = y_i==0 ? ALPHA*log(p1+eps) : s[y_i]*log(1-p1+eps)
    loss    = CE - sum_i v_i / N

Per core: 32768 rows -> 8 batches of 4096 rows, rows packed 32 per
partition (fully linear 1MiB DMAs). Per batch only 4 instructions:
dma_in -> exp(ACT) -> reduce(DVE) -> ln(ACT into a persistent output
buffer). One 128KiB DMA out at the end.
"""

import sys

import numpy as np

if "/opt/trn_rl_repo" not in sys.path:
    sys.path.insert(0, "/opt/trn_rl_repo")

N_CORES = 8
N = 262144
C = 128  # classes
M = N // N_CORES  # rows per core
P = 128  # SBUF partitions
KB = 32  # rows per partition per batch
BATCH_ROWS = P * KB  # 4096
NB = M // BATCH_ROWS  # 8 batches per core
ALPHA = 0.5
BETA = 0.5
EPS = 1e-9

_CACHE: dict = {}


def _build_nc():
    import concourse.bacc as bacc
    import concourse.mybir as mybir
    import concourse.tile as tile

    f16 = mybir.dt.float16
    f32 = mybir.dt.float32
    Ln = mybir.ActivationFunctionType.Ln
    Exp = mybir.ActivationFunctionType.Exp

    nc = bacc.Bacc(
        "TRN2", target_bir_lowering=False, debug=False, num_devices=N_CORES
    )
    y = nc.dram_tensor("y_pred", [M, C], f16, kind="ExternalInput").ap()
    out = nc.dram_tensor("out", [P, NB, KB], f32, kind="ExternalOutput").ap()

    # row(b, p, k) = b*4096 + p*32 + k  (8KB contiguous per partition/batch)
    y4 = y.rearrange("(b p k) c -> b p k c", b=NB, p=P, k=KB)

    with tile.TileContext(nc) as tc:
        with (
            tc.tile_pool(name="persist", bufs=1) as persist,
            tc.tile_pool(name="work", bufs=3) as work,
        ):
            obuf = persist.tile([P, NB, KB], f32)
            for b in range(NB):
                T = work.tile([P, KB, C], f16)
                nc.sync.dma_start(T[:], y4[b])
                E = work.tile([P, KB, C], f16)
                nc.scalar.activation(E[:], T[:], Exp)
                se = work.tile([P, KB], f32)
                nc.vector.reduce_sum(se[:], E[:], axis=mybir.AxisListType.X)
                nc.scalar.activation(obuf[:, b, :], se[:], Ln)
            nc.sync.dma_start(out[:], obuf[:])

    nc.finalize()
    return nc


def _get_nc():
    if "nc" not in _CACHE:
        _CACHE["nc"] = _build_nc()
    return _CACHE["nc"]


def _make_in_maps(y_pred: np.ndarray):
    y16 = np.asarray(y_pred).astype(np.float16)
    return [{"y_pred": np.ascontiguousarray(y16[c * M : (c + 1) * M])} for c in range(N_CORES)]


def _run(in_maps, trace=False, **kwargs):
    from concourse.bass_utils import run_bass_kernel_spmd

    nc = _get_nc()
    return run_bass_kernel_spmd(
        nc, in_maps, list(range(N_CORES)), trace=trace, **kwargs
    )


def _combine(results, y_pred: np.ndarray, y_true: np.ndarray) -> np.ndarray:
    yp = np.asarray(y_pred)
    yt = np.asarray(y_true).reshape(-1).astype(np.int64)

    # Per-row logsumexp from the device: out[p, b, k] is row b*4096 + p*32 + k.
    lse = np.empty(N, dtype=np.float64)
    for c in range(N_CORES):
        o = results[c]["out"].astype(np.float64)  # [P, NB, KB]
        lse[c * M : (c + 1) * M] = o.transpose(1, 0, 2).reshape(M)

    picked = np.take_along_axis(yp, yt[:, None], axis=1).reshape(-1).astype(np.float64)
    ce = -(picked.sum() - lse.sum()) / N

    p1 = np.exp(yp[:, 0].astype(np.float64) - lse)
    lp = np.log(p1 + EPS)
    lq = np.log((1.0 + EPS) - p1)
    nj = np.bincount(yt, minlength=C).astype(np.float64)
    s = BETA * (1.0 - nj / (N - nj[0]))
    v = np.where(yt == 0, ALPHA * lp, s[yt] * lq)
    loss = ce - v.sum() / N
    return np.asarray(loss, dtype=np.float32)


def kernel(y_pred: np.ndarray, y_true: np.ndarray) -> np.ndarray:
    in_maps = _make_in_maps(y_pred)
    res = _run(in_maps, trace=False)
    return _combine(res.results, y_pred, y_true)


# revision 6
# speedup vs baseline: 4.5386x; 1.1856x over previous
"""Trainium2 Bass kernel for nn_CustomLoss_19061064859882.

loss = CE(y_pred, y_true) - penalty/N, where the penalty uses
p1 = softmax(y_pred)[:, 0] and per-class weights from the label histogram.

Device/host split: everything that is O(N*C) transcendental work — the
per-row logsumexp over the 128 classes — runs on the 8 NeuronCores
(data-parallel over rows, fp16 on the wire, exp on ScalarE + row-reduce on
VectorE). The remaining O(N) bookkeeping (picked-logit gather, label
bincount, per-class weighted sums, final scalar) is cheap vectorized numpy
on the host, done in float64:

    lse_i   = log(sum_c exp(y_pred[i, c]))          # device
    CE      = -(sum_i y_pred[i, y_i] - sum_i lse_i)/N
    p1_i    = exp(y_pred[i, 0] - lse_i)
    v_i     = y_i==0 ? ALPHA*log(p1+eps) : s[y_i]*log(1-p1+eps)
    loss    = CE - sum_i v_i / N

Per core: 32768 rows -> 8 batches of 4096 rows, rows packed 32 per
partition (fully linear 1MiB DMAs). Per batch only 4 instructions:
dma_in -> exp(ACT) -> reduce(DVE) -> ln(ACT into a persistent output
buffer). One 128KiB DMA out at the end.
"""

import sys

import numpy as np

if "/opt/trn_rl_repo" not in sys.path:
    sys.path.insert(0, "/opt/trn_rl_repo")

N_CORES = 8
N = 262144
C = 128  # classes
M = N // N_CORES  # rows per core
P = 128  # SBUF partitions
KB = 32  # rows per partition per batch
BATCH_ROWS = P * KB  # 4096
NB = M // BATCH_ROWS  # 8 batches per core
ALPHA = 0.5
BETA = 0.5
EPS = 1e-9

_CACHE: dict = {}


def _build_nc():
    import concourse.bacc as bacc
    import concourse.mybir as mybir
    import concourse.tile as tile

    f16 = mybir.dt.float16
    f32 = mybir.dt.float32
    Ln = mybir.ActivationFunctionType.Ln
    Exp = mybir.ActivationFunctionType.Exp

    nc = bacc.Bacc(
        "TRN2", target_bir_lowering=False, debug=False, num_devices=N_CORES
    )

    # Exp and Ln live in different default table-sets, so bacc would emit an
    # ACT_TABLE_LOAD (~2.7us) at every Exp<->Ln transition. Strip them from
    # every set except the one that holds both, so a single load serves the
    # whole kernel. (get_activation_tables is functools.cache'd; mutating the
    # returned sets is how we reach bacc's insert_act_table_loads pass.)
    import concourse.hw_specs as hw_specs

    tabs = hw_specs.get_activation_tables(nc.m.arch)
    if "natural_log_exp_and_others" in tabs:
        for name, funcs in tabs.items():
            if name != "natural_log_exp_and_others":
                funcs.discard(Exp)
                funcs.discard(Ln)

    y = nc.dram_tensor("y_pred", [M, C], f16, kind="ExternalInput").ap()
    out = nc.dram_tensor("out", [P, NB, KB], f32, kind="ExternalOutput").ap()

    # row(b, p, k) = b*4096 + p*32 + k  (8KB contiguous per partition/batch)
    y4 = y.rearrange("(b p k) c -> b p k c", b=NB, p=P, k=KB)

    with tile.TileContext(nc) as tc:
        with (
            tc.tile_pool(name="persist", bufs=1) as persist,
            tc.tile_pool(name="work", bufs=3) as work,
        ):
            obuf = persist.tile([P, NB, KB], f32)
            for b in range(NB):
                T = work.tile([P, KB, C], f16)
                nc.sync.dma_start(T[:], y4[b])
                E = work.tile([P, KB, C], f16)
                nc.scalar.activation(E[:], T[:], Exp)
                # Pairwise halving on GpSimd (otherwise idle), then the
                # fp16 X-reduce on Vector runs on half the elements.
                H = work.tile([P, KB, C // 2], f16)
                nc.gpsimd.tensor_add(H[:], E[:, :, 0 : C // 2], E[:, :, C // 2 : C])
                se = work.tile([P, KB], f32)
                nc.vector.reduce_sum(se[:], H[:], axis=mybir.AxisListType.X)
                nc.scalar.activation(obuf[:, b, :], se[:], Ln)
            nc.sync.dma_start(out[:], obuf[:])

    nc.finalize()
    return nc


def _get_nc():
    if "nc" not in _CACHE:
        _CACHE["nc"] = _build_nc()
    return _CACHE["nc"]


def _make_in_maps(y_pred: np.ndarray):
    y16 = np.asarray(y_pred).astype(np.float16)
    return [{"y_pred": np.ascontiguousarray(y16[c * M : (c + 1) * M])} for c in range(N_CORES)]


def _run(in_maps, trace=False, **kwargs):
    from concourse.bass_utils import run_bass_kernel_spmd

    nc = _get_nc()
    return run_bass_kernel_spmd(
        nc, in_maps, list(range(N_CORES)), trace=trace, **kwargs
    )


def _combine(results, y_pred: np.ndarray, y_true: np.ndarray) -> np.ndarray:
    yp = np.asarray(y_pred)
    yt = np.asarray(y_true).reshape(-1).astype(np.int64)

    # Per-row logsumexp from the device: out[p, b, k] is row b*4096 + p*32 + k.
    lse = np.empty(N, dtype=np.float64)
    for c in range(N_CORES):
        o = results[c]["out"].astype(np.float64)  # [P, NB, KB]
        lse[c * M : (c + 1) * M] = o.transpose(1, 0, 2).reshape(M)

    picked = np.take_along_axis(yp, yt[:, None], axis=1).reshape(-1).astype(np.float64)
    ce = -(picked.sum() - lse.sum()) / N

    p1 = np.exp(yp[:, 0].astype(np.float64) - lse)
    lp = np.log(p1 + EPS)
    lq = np.log((1.0 + EPS) - p1)
    nj = np.bincount(yt, minlength=C).astype(np.float64)
    s = BETA * (1.0 - nj / (N - nj[0]))
    v = np.where(yt == 0, ALPHA * lp, s[yt] * lq)
    loss = ce - v.sum() / N
    return np.asarray(loss, dtype=np.float32)


def kernel(y_pred: np.ndarray, y_true: np.ndarray) -> np.ndarray:
    in_maps = _make_in_maps(y_pred)
    res = _run(in_maps, trace=False)
    return _combine(res.results, y_pred, y_true)
